# revision 17
# baseline (speedup 1.0000x reference)
"""Trainium2 Bass kernel for the Critic (gnn_message_passing) problem.

Math (per sample b):
  wg   = W_w @ g + W_b                                  [32]
  ul_l = U_w @ x_l + U_b                                [32]  (never materialized)
  score_l = lrelu(a1.wg + a2.ul_l + att_b) = lrelu(x_l . v + c_b)
        where v = U_w^T a2  (128-vec),  c_b = a1.wg + att_b + U_b.a2
  score_g = lrelu((a1+a2).wg + att_b)
  total = score_g + sum_l score_l
  l_part = (U_w @ m_b + U_b * s_b) / total   with m_b = sum_l score_l x_l, s_b = sum_l score_l
  g_part = (score_g / total) * wg
  sa = [relu(g_part); relu(l_part); action]            [128]
  q_h = l3 @ relu(l2 @ relu(l1 @ sa + b1) + b2) + b3   (two heads)

Layout (per core, pure data parallel x8, B_LOC = 512 samples):
  - local_states streamed in 32 fp32 chunks of 16 samples, [128 part, 25*128]:
    partition p holds 25 CONSECUTIVE tokens of sample p//8 (12.8KB contiguous
    descriptors). ACT makes an fp16 copy for the PE m-pass.
  - t = x.v stays fully fp32 (score errors amplify ~40x downstream): V-lane
    chunks run 25 fused TTR custom ops on DVE (c folded in as the accum
    init); G-lane chunks run one big GPSIMD multiply + one segmented DVE
    tensor_reduce + ACT bias add. Ratio tuned to balance DVE vs GPSIMD.
  - score split hi+lo (fp16 pair carries ~22 mantissa bits); m accumulated
    on PE with the x16 tile stationary and scorem [128, 2*16] moving:
    mT_chunk [128 feat, 2*16] in PSUM, folded hi+lo on copy-out.
  - s via two m16-stationary matmuls + ACT accumulate into s_colT [16, 32];
    one SBUF->SBUF DMA flattens to s_row [1, 512] after the loop.
  - Head MLPs run feature-major (transposed activations) on PE (fp32).
"""
import os
import sys

sys.path.insert(0, "/opt/trn_rl_repo")

from contextlib import ExitStack

import numpy as np

import concourse.bass as bass
import concourse.tile as tile
from concourse import bacc
from concourse import mybir
from concourse.dve_ops import TENSOR_TENSOR_REDUCE as CUSTOM_TTR

F32 = mybir.dt.float32
F16 = mybir.dt.float16
AF = mybir.AluOpType
IDENT = mybir.ActivationFunctionType.Identity
RELU = mybir.ActivationFunctionType.Relu

G_DIM, L_DIM, A_DIM, HID = 256, 128, 64, 32
B, L = 4096, 200
NCORES = 8
B_LOC = B // NCORES          # 512 samples per core
J = 25                       # tokens per partition per chunk
SPC = 16                     # samples per chunk (128 partitions / 8 per sample)
PPS = L // J                 # partitions per sample = 8
NCHUNK = B_LOC // SPC        # 32 chunks
NB = B_LOC // 128            # 128-sample blocks
V_LANES = 7                  # of every 16 chunks, this many use the DVE TTR lane


def build_bass(b_loc=B_LOC):
    tok = b_loc * L
    nc = bacc.Bacc()

    ls = nc.dram_tensor("local_states", [tok, L_DIM], F32, kind="ExternalInput")
    gs = nc.dram_tensor("global_states", [b_loc, G_DIM], F32, kind="ExternalInput")
    ac = nc.dram_tensor("actions", [b_loc, A_DIM], F32, kind="ExternalInput")
    Ww = nc.dram_tensor("W_w", [HID, G_DIM], F32, kind="ExternalInput")
    Wb = nc.dram_tensor("W_b", [HID], F32, kind="ExternalInput")
    Uw = nc.dram_tensor("U_w", [HID, L_DIM], F32, kind="ExternalInput")
    Ub = nc.dram_tensor("U_b", [HID], F32, kind="ExternalInput")
    attw = nc.dram_tensor("att_w", [1, 2 * HID], F32, kind="ExternalInput")
    attb = nc.dram_tensor("att_b", [1], F32, kind="ExternalInput")
    heads = []
    for h, names in enumerate((("l1", "l2", "l3"), ("l4", "l5", "l6"))):
        w1 = nc.dram_tensor(f"{names[0]}_w", [256, 128], F32, kind="ExternalInput")
        b1 = nc.dram_tensor(f"{names[0]}_b", [256], F32, kind="ExternalInput")
        w2 = nc.dram_tensor(f"{names[1]}_w", [256, 256], F32, kind="ExternalInput")
        b2 = nc.dram_tensor(f"{names[1]}_b", [256], F32, kind="ExternalInput")
        w3 = nc.dram_tensor(f"{names[2]}_w", [1, 256], F32, kind="ExternalInput")
        b3 = nc.dram_tensor(f"{names[2]}_b", [1], F32, kind="ExternalInput")
        heads.append((w1, b1, w2, b2, w3, b3))
    m16_d = nc.dram_tensor("m16", [128, SPC], F16, kind="ExternalInput")
    esel_d = nc.dram_tensor("esel", [128, PPS * 128], F32, kind="ExternalInput")
    out_d = nc.dram_tensor("out", [2, b_loc], F32, kind="ExternalOutput")

    with tile.TileContext(nc) as tc, ExitStack() as ctx:
        P = ctx.enter_context(tc.tile_pool(name="persist", bufs=1))
        scratch = ctx.enter_context(tc.tile_pool(name="scratch", bufs=2))
        ctxA = ctx.enter_context(ExitStack())
        ps_t = ctxA.enter_context(tc.tile_pool(name="ps_t", bufs=2, space="PSUM"))

        # ---------------- Phase A (lean): just what the stream needs --------
        from concourse.masks import make_identity

        ident = P.tile([128, 128], F32, tag="ident")
        make_identity(nc, ident[:, :])
        ones_row = P.tile([1, 128], F32, tag="onesr")
        nc.vector.memset(ones_row[:, :], 1.0)
        m16_sb = P.tile([128, SPC], F16, tag="m16")
        nc.sync.dma_start(m16_sb[:, :], m16_d[:, :])
        esel = P.tile([128, PPS * 128], F32, tag="esel")
        nc.sync.dma_start(esel[:, :], esel_d[:, :])

        def transpose_to_sbuf(dst_ap, src_ap):
            pp, ff = src_ap.shape
            t_ps = ps_t.tile([128, 128], F32, tag="tps")
            nc.tensor.transpose(t_ps[0:ff, 0:pp], src_ap, ident[0:pp, 0:pp])
            nc.scalar.copy(dst_ap, t_ps[0:ff, 0:pp])

        # small weights
        Ww_sb = P.tile([HID, G_DIM], F32, tag="Ww")
        nc.sync.dma_start(Ww_sb[:, :], Ww[:, :])
        Wb_sb = P.tile([HID, 1], F32, tag="Wb")
        nc.sync.dma_start(Wb_sb[:, :], Wb[:][:, None])
        Uw_sb = P.tile([HID, L_DIM], F32, tag="Uw")
        nc.sync.dma_start(Uw_sb[:, :], Uw[:, :])
        Ub_col = P.tile([HID, 1], F32, tag="Ubc")
        nc.sync.dma_start(Ub_col[:, :], Ub[:][:, None])
        Ub_row = P.tile([1, HID], F32, tag="Ubr")
        nc.sync.dma_start(Ub_row[:, :], Ub[:][None, :])
        a1_sb = P.tile([HID, 1], F32, tag="a1")
        nc.sync.dma_start(a1_sb[:, :], attw[0, 0:HID][:, None])
        a2_sb = P.tile([HID, 1], F32, tag="a2")
        nc.sync.dma_start(a2_sb[:, :], attw[0, HID:2 * HID][:, None])
        attb_sb = P.tile([1, 1], F32, tag="attb")
        nc.sync.dma_start(attb_sb[:, :], attb[:][None, :])

        # global_states (kept resident; also reused for gT transposes later)
        g_nat = []
        for bb in range(NB):
            g = P.tile([128, G_DIM], F32, tag=f"gnat{bb}")
            nc.sync.dma_start(g[:, :], gs[bb * 128:(bb + 1) * 128, :])
            g_nat.append(g)

        # v_rep [128, 128] fp32 = ones (x) (a2^T U_w)
        v_ps = ps_t.tile([1, L_DIM], F32, tag="tps")
        nc.tensor.matmul(out=v_ps[:, :], lhsT=a2_sb[:, :], rhs=Uw_sb[:, :])
        v_row = P.tile([1, L_DIM], F32, tag="vrow")
        nc.scalar.copy(v_row[:, :], v_ps[:, :])
        vrep_ps = ps_t.tile([128, 128], F32, tag="tps")
        nc.tensor.matmul(out=vrep_ps[:, :], lhsT=ones_row[:, :], rhs=v_row[:, :])
        v_rep = P.tile([128, 128], F32, tag="vrep")
        nc.scalar.copy(v_rep[:, :], vrep_ps[:, :])

        # c_col4 [128, NB]: c (minus const) per sample, sample-partition,
        # via u = W_w^T a1 then per-block STT free-dim accumulation.
        u_ps = ps_t.tile([1, G_DIM], F32, tag="tps")
        nc.tensor.matmul(out=u_ps[:, :], lhsT=a1_sb[:, :], rhs=Ww_sb[:, :])
        u_row = P.tile([1, G_DIM], F32, tag="urow")
        nc.scalar.copy(u_row[:, :], u_ps[:, :])
        urep_ps = ps_t.tile([128, G_DIM], F32, tag="tps")
        nc.tensor.matmul(out=urep_ps[:, :], lhsT=ones_row[:, :], rhs=u_row[:, :])
        u_rep = P.tile([128, G_DIM], F32, tag="urep")
        nc.scalar.copy(u_rep[:, :], urep_ps[:, :])
        c_col4 = P.tile([128, NB], F32, tag="ccol4")
        junkA = scratch.tile([128, G_DIM], F32, tag="junkA")
        for bb in range(NB):
            nc.vector._custom_dve(
                CUSTOM_TTR, out=junkA[:, :], in0=g_nat[bb][:, :],
                in1=u_rep[:, :], s0=0.0, s1=1.0,
                accum_out=c_col4[:, bb:bb + 1])

        # cconst = U_b.a2 + att_b + a1.W_b, broadcast to 128 partitions
        uba2_ps = ps_t.tile([1, 1], F32, tag="tps")
        nc.tensor.matmul(out=uba2_ps[:, :], lhsT=Ub_col[:, :], rhs=a2_sb[:, :],
                         start=True, stop=False, skip_group_check=True)
        nc.tensor.matmul(out=uba2_ps[:, :], lhsT=Wb_sb[:, :], rhs=a1_sb[:, :],
                         start=False, stop=True, skip_group_check=True)
        cconst = P.tile([1, 1], F32, tag="cconst")
        nc.vector.tensor_tensor(out=cconst[:, :], in0=uba2_ps[:, :],
                                in1=attb_sb[:, :], op=AF.add)
        cc128_ps = ps_t.tile([128, 1], F32, tag="tps")
        nc.tensor.matmul(out=cc128_ps[:, :], lhsT=ones_row[0:1, :],
                         rhs=cconst[:, :])
        cc128 = P.tile([128, 1], F32, tag="cc128")
        nc.scalar.copy(cc128[:, :], cc128_ps[:, :])

        # c_all [128, NCHUNK]: column ch holds c[ch*16 + p//8].
        # esel[:, r*128:(r+1)*128] is E_r with E_r[q, p] = ind(q == r*16+p//8);
        # c_all[:, r::PPS] = E_r^T @ c_col4.
        call_ps = ps_t.tile([128, NCHUNK], F32, tag="tps")
        for r in range(PPS):
            nc.tensor.matmul(out=call_ps[:, r:NCHUNK:PPS],
                             lhsT=esel[:, r * 128:(r + 1) * 128],
                             rhs=c_col4[:, :], skip_group_check=True)
        c_all = P.tile([128, NCHUNK], F32, tag="call")
        nc.scalar.activation(c_all[:, :], call_ps[:, :], IDENT,
                             bias=cc128[:, :])

        # ---------------- Phase B: main token stream ----------------
        ctxB = ctx.enter_context(ExitStack())
        xpool = ctx.enter_context(tc.tile_pool(name="xchunk", bufs=3))
        x16pool = ctx.enter_context(tc.tile_pool(name="x16", bufs=2))
        ppool = ctx.enter_context(tc.tile_pool(name="prod", bufs=2))
        tpool = ctx.enter_context(tc.tile_pool(name="tbuf", bufs=2))
        jpool = ctx.enter_context(tc.tile_pool(name="junk", bufs=2))
        smpool = ctx.enter_context(tc.tile_pool(name="scorem", bufs=2))
        ps_m = ctxB.enter_context(tc.tile_pool(name="ps_m", bufs=2, space="PSUM"))
        ps_s = ctxB.enter_context(tc.tile_pool(name="ps_s", bufs=2, space="PSUM"))

        mT = P.tile([L_DIM, b_loc], F32, tag="mT")
        s_colT = P.tile([SPC, NCHUNK], F32, tag="scolT")

        for ch in range(NCHUNK):
            use_v = (ch % 16) < V_LANES
            x_ch = xpool.tile([128, J * 128], F32, tag="xch")
            src = ls[ch * J * 128:(ch + 1) * J * 128, :]
            nc.sync.dma_start(
                x_ch[:, :], src.rearrange("(p j) d -> p (j d)", p=128))
            x16 = x16pool.tile([128, J * 128], F16, tag="x16")
            nc.scalar.copy(x16[:, :], x_ch[:, :])

            # z[p, j] = sum_d x[p,j,d]*v[d] + c  (all fp32)
            z = tpool.tile([128, J], F32, tag="z")
            if use_v:
                junk = jpool.tile([128, 128], F32, tag="jk")
                for j in range(J):
                    nc.vector._custom_dve(
                        CUSTOM_TTR, out=junk[:, :],
                        in0=x_ch[:, j * 128:(j + 1) * 128], in1=v_rep[:, :],
                        s0=c_all[:, ch:ch + 1], s1=1.0,
                        accum_out=z[:, j:j + 1])
            else:
                prod = ppool.tile([128, J * 128], F32, tag="pr")
                nc.gpsimd.tensor_tensor(
                    out=prod[:, :].rearrange("p (j d) -> p j d", d=128),
                    in0=x_ch[:, :].rearrange("p (j d) -> p j d", d=128),
                    in1=v_rep[:, None, :].broadcast_to((128, J, 128)),
                    op=AF.mult)
                t_raw = tpool.tile([128, J], F32, tag="traw")
                nc.vector.tensor_reduce(
                    out=t_raw[:, :],
                    in_=prod[:, :].rearrange("p (j d) -> p j d", d=128),
                    axis=mybir.AxisListType.X, op=AF.add)
                nc.scalar.activation(z[:, :], t_raw[:, :], IDENT,
                                     bias=c_all[:, ch:ch + 1])

            score = tpool.tile([128, J], F32, tag="sc")
            nc.vector.scalar_tensor_tensor(
                out=score[:, :], in0=z[:, :], scalar=0.01, in1=z[:, :],
                op0=AF.mult, op1=AF.max)
            # split score = hi + lo (fp16 pair ~= 22 mantissa bits)
            hi = tpool.tile([128, J], F16, tag="hi")
            nc.scalar.copy(hi[:, :], score[:, :])
            lo = tpool.tile([128, J], F16, tag="lo")
            nc.vector.tensor_tensor(out=lo[:, :], in0=score[:, :],
                                    in1=hi[:, :], op=AF.subtract)
            scorem = smpool.tile([128, J * 2 * SPC], F16, tag="sm")
            sm3 = scorem[:, :].rearrange("p (j t s) -> p j t s", t=2, s=SPC)
            nc.gpsimd.tensor_tensor(
                out=sm3[:, :, 0, :],
                in0=hi[:, :, None].broadcast_to((128, J, SPC)),
                in1=m16_sb[:, None, :].broadcast_to((128, J, SPC)),
                op=AF.mult)
            nc.gpsimd.tensor_tensor(
                out=sm3[:, :, 1, :],
                in0=lo[:, :, None].broadcast_to((128, J, SPC)),
                in1=m16_sb[:, None, :].broadcast_to((128, J, SPC)),
                op=AF.mult)

            # mT_chunk [128 feat, 2*16] accumulated over the 25 tiles
            mT_ps = ps_m.tile([L_DIM, 2 * SPC], F32, tag="mps")
            for j in range(J):
                nc.tensor.matmul(out=mT_ps[:, :],
                                 lhsT=x16[:, j * 128:(j + 1) * 128],
                                 rhs=scorem[:, j * 2 * SPC:(j + 1) * 2 * SPC],
                                 start=(j == 0), stop=(j == J - 1))
            nc.scalar.copy(mT[:, ch * SPC:(ch + 1) * SPC], mT_ps[:, 0:SPC])
            nc.vector.tensor_tensor(out=mT[:, ch * SPC:(ch + 1) * SPC],
                                    in0=mT_ps[:, SPC:2 * SPC],
                                    in1=mT[:, ch * SPC:(ch + 1) * SPC],
                                    op=AF.add)

            # s[s] = sum_{p,j} (hi+lo)[p,j]*ind(p//8==s)  -> s_colT[:, ch]
            s_ps = ps_s.tile([SPC, J], F32, tag="sps")
            nc.tensor.matmul(out=s_ps[:, :], lhsT=m16_sb[:, :], rhs=hi[:, :],
                             start=True, stop=False)
            nc.tensor.matmul(out=s_ps[:, :], lhsT=m16_sb[:, :], rhs=lo[:, :],
                             start=False, stop=True)
            junk_s = jpool.tile([SPC, J], F32, tag="jks")
            nc.scalar.activation(junk_s[:, :], s_ps[:, :], IDENT,
                                 accum_out=s_colT[:, ch:ch + 1])

        # flatten s_colT [16, 32] -> s_row [1, 512] (order: sample = ch*16+s)
        # via 16 selection matmuls into one PSUM row (strided col slices)
        srow_ps = ps_s.tile([1, b_loc], F32, tag="srps")
        for s in range(SPC):
            nc.tensor.matmul(out=srow_ps[0:1, s * NCHUNK:(s + 1) * NCHUNK],
                             lhsT=ident[0:SPC, s:s + 1], rhs=s_colT[:, :],
                             skip_group_check=True)
        s_row = P.tile([1, b_loc], F32, tag="srow")
        nc.scalar.copy(
            s_row[0:1, :].rearrange("one (c s) -> one c s", s=SPC),
            srow_ps[0:1, :].rearrange("one (s c) -> one c s", s=SPC))

        ctxB.close()

        # ------------- Phase A tail (overlaps the stream) -------------------
        # gT transposes, wgT, sg_raw, saT/actions, head weights: consumed only
        # by phase C, so they schedule behind the chunk traffic.
        gT = []
        for g in range(G_DIM // 128):
            t = P.tile([128, b_loc], F32, tag=f"gT{g}")
            gT.append(t)
        for bb in range(NB):
            for g in range(G_DIM // 128):
                transpose_to_sbuf(gT[g][:, bb * 128:(bb + 1) * 128],
                                  g_nat[bb][:, g * 128:(g + 1) * 128])
        WwT = []
        for g in range(G_DIM // 128):
            w = P.tile([128, HID], F32, tag=f"WwT{g}")
            transpose_to_sbuf(w[:, :], Ww_sb[:, g * 128:(g + 1) * 128])
            WwT.append(w)
        UwT = P.tile([L_DIM, HID], F32, tag="UwT")
        transpose_to_sbuf(UwT[:, :], Uw_sb[:, :])

        wgT_ps = ps_t.tile([HID, b_loc], F32, tag="tps")
        for g in range(G_DIM // 128):
            nc.tensor.matmul(out=wgT_ps[:, :], lhsT=WwT[g][:, :], rhs=gT[g][:, :],
                             start=(g == 0), stop=(g == G_DIM // 128 - 1))
        wgT = P.tile([HID, b_loc], F32, tag="wgT")
        nc.scalar.activation(wgT[:, :], wgT_ps[:, :], IDENT, bias=Wb_sb[:, :])

        a12 = P.tile([HID, 1], F32, tag="a12")
        nc.vector.tensor_tensor(out=a12[:, :], in0=a1_sb[:, :], in1=a2_sb[:, :],
                                op=AF.add)
        sg_ps = ps_t.tile([1, b_loc], F32, tag="tps")
        nc.tensor.matmul(out=sg_ps[:, :], lhsT=a12[:, :], rhs=wgT[:, :])
        sg_lin = P.tile([1, b_loc], F32, tag="sg_lin")
        nc.scalar.activation(sg_lin[:, :], sg_ps[:, :], IDENT, bias=attb_sb[:, :])
        sg_raw = P.tile([1, b_loc], F32, tag="sg_raw")
        nc.vector.scalar_tensor_tensor(out=sg_raw[:, :], in0=sg_lin[:, :],
                                       scalar=0.01, in1=sg_lin[:, :],
                                       op0=AF.mult, op1=AF.max)

        saT = P.tile([128, b_loc], F32, tag="saT")
        for bb in range(NB):
            a_nat = scratch.tile([128, A_DIM], F32, tag="anat")
            nc.sync.dma_start(a_nat[:, :], ac[bb * 128:(bb + 1) * 128, :])
            transpose_to_sbuf(saT[2 * HID:2 * HID + A_DIM, bb * 128:(bb + 1) * 128],
                              a_nat[:, :])

        head_sb = []
        for (w1, b1, w2, b2, w3, b3) in heads:
            w1_nat = scratch.tile([128, 128], F32, tag="w1nat")
            w1T = P.tile([128, 256], F32, tag=f"w1T{len(head_sb)}")
            for rh in range(2):
                nc.sync.dma_start(w1_nat[:, :], w1[rh * 128:(rh + 1) * 128, :])
                transpose_to_sbuf(w1T[:, rh * 128:(rh + 1) * 128], w1_nat[:, :])
            w2T = [P.tile([128, 256], F32, tag=f"w2T{len(head_sb)}_{kh}",
                          name=f"w2T{len(head_sb)}_{kh}")
                   for kh in range(2)]
            for rh in range(2):
                for kh in range(2):
                    w2_nat = scratch.tile([128, 128], F32, tag="w2nat")
                    nc.sync.dma_start(
                        w2_nat[:, :],
                        w2[rh * 128:(rh + 1) * 128, kh * 128:(kh + 1) * 128])
                    transpose_to_sbuf(w2T[kh][:, rh * 128:(rh + 1) * 128],
                                      w2_nat[:, :])
            w3T = P.tile([128, 2], F32, tag=f"w3T{len(head_sb)}")
            for kh in range(2):
                nc.sync.dma_start(w3T[:, kh:kh + 1],
                                  w3[0, kh * 128:(kh + 1) * 128][:, None])
            b1c = P.tile([128, 2], F32, tag=f"b1c{len(head_sb)}")
            b2c = P.tile([128, 2], F32, tag=f"b2c{len(head_sb)}")
            for rh in range(2):
                nc.sync.dma_start(b1c[:, rh:rh + 1],
                                  b1[rh * 128:(rh + 1) * 128][:, None])
                nc.sync.dma_start(b2c[:, rh:rh + 1],
                                  b2[rh * 128:(rh + 1) * 128][:, None])
            b3c = P.tile([1, 1], F32, tag=f"b3c{len(head_sb)}")
            nc.sync.dma_start(b3c[:, :], b3[:][None, :])
            head_sb.append((w1T, w2T, w3T, b1c, b2c, b3c))

        ctxA.close()

        # ---------------- Phase C: combine + heads ----------------
        ps_c = ctx.enter_context(tc.tile_pool(name="ps_c", bufs=4, space="PSUM"))
        _phase_c(nc, tc, ctx, b_loc, P, scratch, ps_c, sg_raw, s_row,
                 ones_row, UwT, mT, Ub_row, wgT, saT, head_sb, out_d)

    nc.compile()
    return nc


def _phase_c(nc, tc, ctx, b_loc, P, scratch, ps_c, sg_raw, s_row,
             ones_row, UwT, mT, Ub_row, wgT, saT, head_sb, out_d):
    total = P.tile([1, b_loc], F32, tag="total")
    nc.vector.tensor_tensor(out=total[:, :], in0=sg_raw[:, :], in1=s_row[:, :],
                            op=AF.add)
    recip = P.tile([1, b_loc], F32, tag="recip")
    nc.vector.reciprocal_approx_fast(recip[:, :], total[:, :])
    gn_row = P.tile([1, b_loc], F32, tag="gn")
    nc.vector.tensor_tensor(out=gn_row[:, :], in0=sg_raw[:, :], in1=recip[:, :],
                            op=AF.mult)

    r32_ps = ps_c.tile([HID, b_loc], F32, tag="cps")
    nc.tensor.matmul(out=r32_ps[:, :], lhsT=ones_row[0:1, 0:HID], rhs=recip[:, :])
    r32 = P.tile([HID, b_loc], F32, tag="r32")
    nc.scalar.copy(r32[:, :], r32_ps[:, :])
    g32_ps = ps_c.tile([HID, b_loc], F32, tag="cps")
    nc.tensor.matmul(out=g32_ps[:, :], lhsT=ones_row[0:1, 0:HID], rhs=gn_row[:, :])
    g32 = P.tile([HID, b_loc], F32, tag="g32")
    nc.scalar.copy(g32[:, :], g32_ps[:, :])

    lT_ps = ps_c.tile([HID, b_loc], F32, tag="cps")
    nc.tensor.matmul(out=lT_ps[:, :], lhsT=UwT[:, :], rhs=mT[:, :],
                     start=True, stop=False)
    nc.tensor.matmul(out=lT_ps[:, :], lhsT=Ub_row[:, :], rhs=s_row[:, :],
                     start=False, stop=True)

    lnorm = P.tile([HID, b_loc], F32, tag="lnorm")
    nc.vector.tensor_tensor(out=lnorm[:, :], in0=lT_ps[:, :], in1=r32[:, :],
                            op=AF.mult)
    gpart = P.tile([HID, b_loc], F32, tag="gpart")
    nc.vector.tensor_tensor(out=gpart[:, :], in0=wgT[:, :], in1=g32[:, :],
                            op=AF.mult)
    nc.scalar.activation(saT[0:HID, :], gpart[:, :], RELU)
    nc.scalar.activation(saT[HID:2 * HID, :], lnorm[:, :], RELU)

    for h, (w1T, w2T, w3T, b1c, b2c, b3c) in enumerate(head_sb):
        h1 = []
        for rh in range(2):
            h_ps = ps_c.tile([128, b_loc], F32, tag="cps")
            nc.tensor.matmul(out=h_ps[:, :], lhsT=w1T[:, rh * 128:(rh + 1) * 128],
                             rhs=saT[:, :])
            h_sb = scratch.tile([128, b_loc], F32, tag="h1sb")
            nc.scalar.activation(h_sb[:, :], h_ps[:, :], RELU,
                                 bias=b1c[:, rh:rh + 1])
            h1.append(h_sb)
        h2 = []
        for rh in range(2):
            h_ps = ps_c.tile([128, b_loc], F32, tag="cps")
            for kh in range(2):
                nc.tensor.matmul(out=h_ps[:, :],
                                 lhsT=w2T[kh][:, rh * 128:(rh + 1) * 128],
                                 rhs=h1[kh][:, :],
                                 start=(kh == 0), stop=(kh == 1))
            h_sb = scratch.tile([128, b_loc], F32, tag="h2sb")
            nc.scalar.activation(h_sb[:, :], h_ps[:, :], RELU,
                                 bias=b2c[:, rh:rh + 1])
            h2.append(h_sb)
        q_ps = ps_c.tile([1, b_loc], F32, tag="cps")
        for kh in range(2):
            nc.tensor.matmul(out=q_ps[:, :], lhsT=w3T[:, kh:kh + 1],
                             rhs=h2[kh][:, :], start=(kh == 0), stop=(kh == 1))
        q_row = scratch.tile([1, b_loc], F32, tag="qrow")
        nc.scalar.activation(q_row[:, :], q_ps[:, :], IDENT, bias=b3c[:, :])
        nc.sync.dma_start(out_d[h:h + 1, :], q_row[:, :])


def _make_m16():
    m = np.zeros((128, SPC), np.float16)
    for p in range(128):
        m[p, p // PPS] = 1.0
    return m


def _make_esel():
    e = np.zeros((128, PPS * 128), np.float32)
    for r in range(PPS):
        for p in range(128):
            e[r * SPC + p // PPS, r * 128 + p] = 1.0
    return e


def _shard_inputs(inputs, b_loc=B_LOC):
    """Full inputs -> list of per-core in_maps."""
    m16 = _make_m16()
    esel = _make_esel()
    maps = []
    for c in range(NCORES):
        sl = slice(c * b_loc, (c + 1) * b_loc)
        m = {
            "local_states": np.ascontiguousarray(
                inputs["local_states"][sl].reshape(b_loc * L, L_DIM)),
            "global_states": np.ascontiguousarray(inputs["global_states"][sl]),
            "actions": np.ascontiguousarray(inputs["actions"][sl]),
            "m16": m16,
            "esel": esel,
        }
        for k in ("W_w", "W_b", "U_w", "U_b", "att_b",
                  "l1_w", "l1_b", "l2_w", "l2_b", "l3_w", "l3_b",
                  "l4_w", "l4_b", "l5_w", "l5_b", "l6_w", "l6_b"):
            m[k] = np.ascontiguousarray(np.asarray(inputs[k], np.float32))
        m["att_w"] = np.ascontiguousarray(
            np.asarray(inputs["att_w"], np.float32).reshape(1, 2 * HID))
        maps.append(m)
    return maps


_CACHE = {}


def kernel(**inputs) -> np.ndarray:
    from concourse.bass_utils import run_bass_kernel_spmd

    inputs = {k: np.asarray(v, np.float32) for k, v in inputs.items()}
    if "nc" not in _CACHE:
        _CACHE["nc"] = build_bass()
    nc = _CACHE["nc"]
    maps = _shard_inputs(inputs)
    res = run_bass_kernel_spmd(nc, maps, list(range(NCORES)))
    outs = [res.results[c]["out"] for c in range(NCORES)]  # each [2, B_LOC]
    q = np.concatenate(outs, axis=1)  # [2, B]
    return q.reshape(2, B, 1).astype(np.float32)


# revision 19
# speedup vs baseline: 1.0267x; 1.0267x over previous
"""Trainium2 Bass kernel for the Critic (gnn_message_passing) problem.

Math (per sample b):
  wg   = W_w @ g + W_b                                  [32]
  ul_l = U_w @ x_l + U_b                                [32]  (never materialized)
  score_l = lrelu(a1.wg + a2.ul_l + att_b) = lrelu(x_l . v + c_b)
        where v = U_w^T a2  (128-vec),  c_b = a1.wg + att_b + U_b.a2
  score_g = lrelu((a1+a2).wg + att_b)
  total = score_g + sum_l score_l
  l_part = (U_w @ m_b + U_b * s_b) / total   with m_b = sum_l score_l x_l, s_b = sum_l score_l
  g_part = (score_g / total) * wg
  sa = [relu(g_part); relu(l_part); action]            [128]
  q_h = l3 @ relu(l2 @ relu(l1 @ sa + b1) + b2) + b3   (two heads)

Layout (per core, pure data parallel x8, B_LOC = 512 samples):
  - local_states streamed in 32 fp32 chunks of 16 samples, [128 part, 25*128]:
    partition p holds 25 CONSECUTIVE tokens of sample p//8 (12.8KB contiguous
    descriptors). ACT makes an fp16 copy for the PE m-pass.
  - t = x.v stays fully fp32 (score errors amplify ~40x downstream): V-lane
    chunks run 25 fused TTR custom ops on DVE (c folded in as the accum
    init); G-lane chunks run one big GPSIMD multiply + one segmented DVE
    tensor_reduce + ACT bias add. Ratio tuned to balance DVE vs GPSIMD.
  - score split hi+lo (fp16 pair carries ~22 mantissa bits); m accumulated
    on PE with the x16 tile stationary and scorem [128, 2*16] moving:
    mT_chunk [128 feat, 2*16] in PSUM, folded hi+lo on copy-out.
  - s via two m16-stationary matmuls + ACT accumulate into s_colT [16, 32];
    one SBUF->SBUF DMA flattens to s_row [1, 512] after the loop.
  - Head MLPs run feature-major (transposed activations) on PE (fp32).
"""
import os
import sys

sys.path.insert(0, "/opt/trn_rl_repo")

from contextlib import ExitStack

import numpy as np

import concourse.bass as bass
import concourse.tile as tile
from concourse import bacc
from concourse import mybir
from concourse.dve_ops import TENSOR_TENSOR_REDUCE as CUSTOM_TTR

F32 = mybir.dt.float32
F16 = mybir.dt.float16
AF = mybir.AluOpType
IDENT = mybir.ActivationFunctionType.Identity
RELU = mybir.ActivationFunctionType.Relu

G_DIM, L_DIM, A_DIM, HID = 256, 128, 64, 32
B, L = 4096, 200
NCORES = 8
B_LOC = B // NCORES          # 512 samples per core
J = 25                       # tokens per partition per chunk
SPC = 16                     # samples per chunk (128 partitions / 8 per sample)
PPS = L // J                 # partitions per sample = 8
NCHUNK = B_LOC // SPC        # 32 chunks
NB = B_LOC // 128            # 128-sample blocks
V_LANES = 7                  # of every 16 chunks, this many use the DVE TTR lane


def build_bass(b_loc=B_LOC):
    tok = b_loc * L
    nc = bacc.Bacc()

    ls = nc.dram_tensor("local_states", [tok, L_DIM], F32, kind="ExternalInput")
    gs = nc.dram_tensor("global_states", [b_loc, G_DIM], F32, kind="ExternalInput")
    ac = nc.dram_tensor("actions", [b_loc, A_DIM], F32, kind="ExternalInput")
    Ww = nc.dram_tensor("W_w", [HID, G_DIM], F32, kind="ExternalInput")
    Wb = nc.dram_tensor("W_b", [HID], F32, kind="ExternalInput")
    Uw = nc.dram_tensor("U_w", [HID, L_DIM], F32, kind="ExternalInput")
    Ub = nc.dram_tensor("U_b", [HID], F32, kind="ExternalInput")
    attw = nc.dram_tensor("att_w", [1, 2 * HID], F32, kind="ExternalInput")
    attb = nc.dram_tensor("att_b", [1], F32, kind="ExternalInput")
    heads = []
    for h, names in enumerate((("l1", "l2", "l3"), ("l4", "l5", "l6"))):
        w1 = nc.dram_tensor(f"{names[0]}_w", [256, 128], F32, kind="ExternalInput")
        b1 = nc.dram_tensor(f"{names[0]}_b", [256], F32, kind="ExternalInput")
        w2 = nc.dram_tensor(f"{names[1]}_w", [256, 256], F32, kind="ExternalInput")
        b2 = nc.dram_tensor(f"{names[1]}_b", [256], F32, kind="ExternalInput")
        w3 = nc.dram_tensor(f"{names[2]}_w", [1, 256], F32, kind="ExternalInput")
        b3 = nc.dram_tensor(f"{names[2]}_b", [1], F32, kind="ExternalInput")
        heads.append((w1, b1, w2, b2, w3, b3))
    m16_d = nc.dram_tensor("m16", [128, SPC], F16, kind="ExternalInput")
    esel_d = nc.dram_tensor("esel", [128, PPS * 128], F32, kind="ExternalInput")
    out_d = nc.dram_tensor("out", [2, b_loc], F32, kind="ExternalOutput")

    with tile.TileContext(nc) as tc, ExitStack() as ctx:
        P = ctx.enter_context(tc.tile_pool(name="persist", bufs=1))
        scratch = ctx.enter_context(tc.tile_pool(name="scratch", bufs=2))
        ctxA = ctx.enter_context(ExitStack())
        ps_t = ctxA.enter_context(tc.tile_pool(name="ps_t", bufs=2, space="PSUM"))

        # ---------------- Phase A (lean): just what the stream needs --------
        from concourse.masks import make_identity

        ident = P.tile([128, 128], F32, tag="ident")
        make_identity(nc, ident[:, :])
        ones_row = P.tile([1, 128], F32, tag="onesr")
        nc.vector.memset(ones_row[:, :], 1.0)
        m16_sb = P.tile([128, SPC], F16, tag="m16")
        nc.sync.dma_start(m16_sb[:, :], m16_d[:, :])
        esel = P.tile([128, PPS * 128], F32, tag="esel")
        nc.sync.dma_start(esel[:, :], esel_d[:, :])

        def transpose_to_sbuf(dst_ap, src_ap):
            pp, ff = src_ap.shape
            t_ps = ps_t.tile([128, 128], F32, tag="tps")
            nc.tensor.transpose(t_ps[0:ff, 0:pp], src_ap, ident[0:pp, 0:pp])
            nc.scalar.copy(dst_ap, t_ps[0:ff, 0:pp])

        # small weights
        Ww_sb = P.tile([HID, G_DIM], F32, tag="Ww")
        nc.sync.dma_start(Ww_sb[:, :], Ww[:, :])
        Wb_sb = P.tile([HID, 1], F32, tag="Wb")
        nc.sync.dma_start(Wb_sb[:, :], Wb[:][:, None])
        Uw_sb = P.tile([HID, L_DIM], F32, tag="Uw")
        nc.sync.dma_start(Uw_sb[:, :], Uw[:, :])
        Ub_col = P.tile([HID, 1], F32, tag="Ubc")
        nc.sync.dma_start(Ub_col[:, :], Ub[:][:, None])
        Ub_row = P.tile([1, HID], F32, tag="Ubr")
        nc.sync.dma_start(Ub_row[:, :], Ub[:][None, :])
        a1_sb = P.tile([HID, 1], F32, tag="a1")
        nc.sync.dma_start(a1_sb[:, :], attw[0, 0:HID][:, None])
        a2_sb = P.tile([HID, 1], F32, tag="a2")
        nc.sync.dma_start(a2_sb[:, :], attw[0, HID:2 * HID][:, None])
        attb_sb = P.tile([1, 1], F32, tag="attb")
        nc.sync.dma_start(attb_sb[:, :], attb[:][None, :])

        # global_states (kept resident; also reused for gT transposes later)
        g_nat = []
        for bb in range(NB):
            g = P.tile([128, G_DIM], F32, tag=f"gnat{bb}")
            nc.sync.dma_start(g[:, :], gs[bb * 128:(bb + 1) * 128, :])
            g_nat.append(g)

        # v_rep [128, 128] fp32 = ones (x) (a2^T U_w)
        v_ps = ps_t.tile([1, L_DIM], F32, tag="tps")
        nc.tensor.matmul(out=v_ps[:, :], lhsT=a2_sb[:, :], rhs=Uw_sb[:, :])
        v_row = P.tile([1, L_DIM], F32, tag="vrow")
        nc.scalar.copy(v_row[:, :], v_ps[:, :])
        vrep_ps = ps_t.tile([128, 128], F32, tag="tps")
        nc.tensor.matmul(out=vrep_ps[:, :], lhsT=ones_row[:, :], rhs=v_row[:, :])
        v_rep = P.tile([128, 128], F32, tag="vrep")
        nc.scalar.copy(v_rep[:, :], vrep_ps[:, :])

        # c_col4 [128, NB]: c (minus const) per sample, sample-partition,
        # via u = W_w^T a1 then per-block STT free-dim accumulation.
        u_ps = ps_t.tile([1, G_DIM], F32, tag="tps")
        nc.tensor.matmul(out=u_ps[:, :], lhsT=a1_sb[:, :], rhs=Ww_sb[:, :])
        u_row = P.tile([1, G_DIM], F32, tag="urow")
        nc.scalar.copy(u_row[:, :], u_ps[:, :])
        urep_ps = ps_t.tile([128, G_DIM], F32, tag="tps")
        nc.tensor.matmul(out=urep_ps[:, :], lhsT=ones_row[:, :], rhs=u_row[:, :])
        u_rep = P.tile([128, G_DIM], F32, tag="urep")
        nc.scalar.copy(u_rep[:, :], urep_ps[:, :])
        c_col4 = P.tile([128, NB], F32, tag="ccol4")
        junkA = scratch.tile([128, G_DIM], F32, tag="junkA")
        for bb in range(NB):
            nc.vector._custom_dve(
                CUSTOM_TTR, out=junkA[:, :], in0=g_nat[bb][:, :],
                in1=u_rep[:, :], s0=0.0, s1=1.0,
                accum_out=c_col4[:, bb:bb + 1])

        # cconst = U_b.a2 + att_b + a1.W_b, broadcast to 128 partitions
        uba2_ps = ps_t.tile([1, 1], F32, tag="tps")
        nc.tensor.matmul(out=uba2_ps[:, :], lhsT=Ub_col[:, :], rhs=a2_sb[:, :],
                         start=True, stop=False, skip_group_check=True)
        nc.tensor.matmul(out=uba2_ps[:, :], lhsT=Wb_sb[:, :], rhs=a1_sb[:, :],
                         start=False, stop=True, skip_group_check=True)
        cconst = P.tile([1, 1], F32, tag="cconst")
        nc.vector.tensor_tensor(out=cconst[:, :], in0=uba2_ps[:, :],
                                in1=attb_sb[:, :], op=AF.add)
        cc128_ps = ps_t.tile([128, 1], F32, tag="tps")
        nc.tensor.matmul(out=cc128_ps[:, :], lhsT=ones_row[0:1, :],
                         rhs=cconst[:, :])
        cc128 = P.tile([128, 1], F32, tag="cc128")
        nc.scalar.copy(cc128[:, :], cc128_ps[:, :])

        # c_all [128, NCHUNK]: column ch holds c[ch*16 + p//8].
        # esel[:, r*128:(r+1)*128] is E_r with E_r[q, p] = ind(q == r*16+p//8);
        # c_all[:, r::PPS] = E_r^T @ c_col4.
        call_ps = ps_t.tile([128, NCHUNK], F32, tag="tps")
        for r in range(PPS):
            nc.tensor.matmul(out=call_ps[:, r:NCHUNK:PPS],
                             lhsT=esel[:, r * 128:(r + 1) * 128],
                             rhs=c_col4[:, :], skip_group_check=True)
        c_all = P.tile([128, NCHUNK], F32, tag="call")
        nc.scalar.activation(c_all[:, :], call_ps[:, :], IDENT,
                             bias=cc128[:, :])

        # ---------------- Phase B: main token stream ----------------
        ctxB = ctx.enter_context(ExitStack())
        xpool = ctx.enter_context(tc.tile_pool(name="xchunk", bufs=3))
        x16pool = ctx.enter_context(tc.tile_pool(name="x16", bufs=2))
        ppool = ctx.enter_context(tc.tile_pool(name="prod", bufs=2))
        tpool = ctx.enter_context(tc.tile_pool(name="tbuf", bufs=2))
        jpool = ctx.enter_context(tc.tile_pool(name="junk", bufs=2))
        smpool = ctx.enter_context(tc.tile_pool(name="scorem", bufs=2))
        ps_m = ctxB.enter_context(tc.tile_pool(name="ps_m", bufs=2, space="PSUM"))
        ps_s = ctxB.enter_context(tc.tile_pool(name="ps_s", bufs=2, space="PSUM"))

        mT = P.tile([L_DIM, b_loc], F32, tag="mT")
        s_colT = P.tile([SPC, NCHUNK], F32, tag="scolT")

        for ch in range(NCHUNK):
            use_v = (ch % 3) != 0  # GPSIMD multiply for 2/3 of chunks
            x_ch = xpool.tile([128, J * 128], F32, tag="xch")
            src = ls[ch * J * 128:(ch + 1) * J * 128, :]
            nc.sync.dma_start(
                x_ch[:, :], src.rearrange("(p j) d -> p (j d)", p=128))
            x16 = x16pool.tile([128, J * 128], F16, tag="x16")
            nc.scalar.copy(x16[:, :], x_ch[:, :])

            # z[p, j] = sum_d x[p,j,d]*v[d] + c  (all fp32); the multiply
            # alternates GPSIMD/DVE to balance engine load, reduce on DVE
            prod = ppool.tile([128, J * 128], F32, tag="pr")
            mult_eng = nc.gpsimd if use_v else nc.vector
            mult_eng.tensor_tensor(
                out=prod[:, :].rearrange("p (j d) -> p j d", d=128),
                in0=x_ch[:, :].rearrange("p (j d) -> p j d", d=128),
                in1=v_rep[:, None, :].broadcast_to((128, J, 128)),
                op=AF.mult)
            t_raw = tpool.tile([128, J], F32, tag="traw")
            nc.vector.tensor_reduce(
                out=t_raw[:, :],
                in_=prod[:, :].rearrange("p (j d) -> p j d", d=128),
                axis=mybir.AxisListType.X, op=AF.add)
            z = tpool.tile([128, J], F32, tag="z")
            nc.scalar.activation(z[:, :], t_raw[:, :], IDENT,
                                 bias=c_all[:, ch:ch + 1])

            score = tpool.tile([128, J], F32, tag="sc")
            nc.vector.scalar_tensor_tensor(
                out=score[:, :], in0=z[:, :], scalar=0.01, in1=z[:, :],
                op0=AF.mult, op1=AF.max)
            # split score = hi + lo (fp16 pair ~= 22 mantissa bits)
            hi = tpool.tile([128, J], F16, tag="hi")
            nc.scalar.copy(hi[:, :], score[:, :])
            lo = tpool.tile([128, J], F16, tag="lo")
            nc.vector.tensor_tensor(out=lo[:, :], in0=score[:, :],
                                    in1=hi[:, :], op=AF.subtract)
            scorem = smpool.tile([128, J * 2 * SPC], F16, tag="sm")
            sm3 = scorem[:, :].rearrange("p (j t s) -> p j t s", t=2, s=SPC)
            nc.gpsimd.tensor_tensor(
                out=sm3[:, :, 0, :],
                in0=hi[:, :, None].broadcast_to((128, J, SPC)),
                in1=m16_sb[:, None, :].broadcast_to((128, J, SPC)),
                op=AF.mult)
            nc.gpsimd.tensor_tensor(
                out=sm3[:, :, 1, :],
                in0=lo[:, :, None].broadcast_to((128, J, SPC)),
                in1=m16_sb[:, None, :].broadcast_to((128, J, SPC)),
                op=AF.mult)

            # mT_chunk [128 feat, 2*16] accumulated over the 25 tiles
            mT_ps = ps_m.tile([L_DIM, 2 * SPC], F32, tag="mps")
            for j in range(J):
                nc.tensor.matmul(out=mT_ps[:, :],
                                 lhsT=x16[:, j * 128:(j + 1) * 128],
                                 rhs=scorem[:, j * 2 * SPC:(j + 1) * 2 * SPC],
                                 start=(j == 0), stop=(j == J - 1))
            nc.scalar.copy(mT[:, ch * SPC:(ch + 1) * SPC], mT_ps[:, 0:SPC])
            nc.vector.tensor_tensor(out=mT[:, ch * SPC:(ch + 1) * SPC],
                                    in0=mT_ps[:, SPC:2 * SPC],
                                    in1=mT[:, ch * SPC:(ch + 1) * SPC],
                                    op=AF.add)

            # s[s] = sum_{p,j} (hi+lo)[p,j]*ind(p//8==s)  -> s_colT[:, ch]
            s_ps = ps_s.tile([SPC, J], F32, tag="sps")
            nc.tensor.matmul(out=s_ps[:, :], lhsT=m16_sb[:, :], rhs=hi[:, :],
                             start=True, stop=False)
            nc.tensor.matmul(out=s_ps[:, :], lhsT=m16_sb[:, :], rhs=lo[:, :],
                             start=False, stop=True)
            junk_s = jpool.tile([SPC, J], F32, tag="jks")
            nc.scalar.activation(junk_s[:, :], s_ps[:, :], IDENT,
                                 accum_out=s_colT[:, ch:ch + 1])

        # flatten s_colT [16, 32] -> s_row [1, 512] (order: sample = ch*16+s)
        # via 16 selection matmuls into one PSUM row (strided col slices)
        srow_ps = ps_s.tile([1, b_loc], F32, tag="srps")
        for s in range(SPC):
            nc.tensor.matmul(out=srow_ps[0:1, s * NCHUNK:(s + 1) * NCHUNK],
                             lhsT=ident[0:SPC, s:s + 1], rhs=s_colT[:, :],
                             skip_group_check=True)
        s_row = P.tile([1, b_loc], F32, tag="srow")
        nc.scalar.copy(
            s_row[0:1, :].rearrange("one (c s) -> one c s", s=SPC),
            srow_ps[0:1, :].rearrange("one (s c) -> one c s", s=SPC))

        ctxB.close()

        # ------------- Phase A tail (overlaps the stream) -------------------
        # gT transposes, wgT, sg_raw, saT/actions, head weights: consumed only
        # by phase C, so they schedule behind the chunk traffic.
        gT = []
        for g in range(G_DIM // 128):
            t = P.tile([128, b_loc], F32, tag=f"gT{g}")
            gT.append(t)
        for bb in range(NB):
            for g in range(G_DIM // 128):
                transpose_to_sbuf(gT[g][:, bb * 128:(bb + 1) * 128],
                                  g_nat[bb][:, g * 128:(g + 1) * 128])
        WwT = []
        for g in range(G_DIM // 128):
            w = P.tile([128, HID], F32, tag=f"WwT{g}")
            transpose_to_sbuf(w[:, :], Ww_sb[:, g * 128:(g + 1) * 128])
            WwT.append(w)
        UwT = P.tile([L_DIM, HID], F32, tag="UwT")
        transpose_to_sbuf(UwT[:, :], Uw_sb[:, :])

        wgT_ps = ps_t.tile([HID, b_loc], F32, tag="tps")
        for g in range(G_DIM // 128):
            nc.tensor.matmul(out=wgT_ps[:, :], lhsT=WwT[g][:, :], rhs=gT[g][:, :],
                             start=(g == 0), stop=(g == G_DIM // 128 - 1))
        wgT = P.tile([HID, b_loc], F32, tag="wgT")
        nc.scalar.activation(wgT[:, :], wgT_ps[:, :], IDENT, bias=Wb_sb[:, :])

        a12 = P.tile([HID, 1], F32, tag="a12")
        nc.vector.tensor_tensor(out=a12[:, :], in0=a1_sb[:, :], in1=a2_sb[:, :],
                                op=AF.add)
        sg_ps = ps_t.tile([1, b_loc], F32, tag="tps")
        nc.tensor.matmul(out=sg_ps[:, :], lhsT=a12[:, :], rhs=wgT[:, :])
        sg_lin = P.tile([1, b_loc], F32, tag="sg_lin")
        nc.scalar.activation(sg_lin[:, :], sg_ps[:, :], IDENT, bias=attb_sb[:, :])
        sg_raw = P.tile([1, b_loc], F32, tag="sg_raw")
        nc.vector.scalar_tensor_tensor(out=sg_raw[:, :], in0=sg_lin[:, :],
                                       scalar=0.01, in1=sg_lin[:, :],
                                       op0=AF.mult, op1=AF.max)

        saT = P.tile([128, b_loc], F32, tag="saT")
        for bb in range(NB):
            a_nat = scratch.tile([128, A_DIM], F32, tag="anat")
            nc.sync.dma_start(a_nat[:, :], ac[bb * 128:(bb + 1) * 128, :])
            transpose_to_sbuf(saT[2 * HID:2 * HID + A_DIM, bb * 128:(bb + 1) * 128],
                              a_nat[:, :])

        head_sb = []
        for (w1, b1, w2, b2, w3, b3) in heads:
            w1_nat = scratch.tile([128, 128], F32, tag="w1nat")
            w1T = P.tile([128, 256], F32, tag=f"w1T{len(head_sb)}")
            for rh in range(2):
                nc.sync.dma_start(w1_nat[:, :], w1[rh * 128:(rh + 1) * 128, :])
                transpose_to_sbuf(w1T[:, rh * 128:(rh + 1) * 128], w1_nat[:, :])
            w2T = [P.tile([128, 256], F32, tag=f"w2T{len(head_sb)}_{kh}",
                          name=f"w2T{len(head_sb)}_{kh}")
                   for kh in range(2)]
            for rh in range(2):
                for kh in range(2):
                    w2_nat = scratch.tile([128, 128], F32, tag="w2nat")
                    nc.sync.dma_start(
                        w2_nat[:, :],
                        w2[rh * 128:(rh + 1) * 128, kh * 128:(kh + 1) * 128])
                    transpose_to_sbuf(w2T[kh][:, rh * 128:(rh + 1) * 128],
                                      w2_nat[:, :])
            w3T = P.tile([128, 2], F32, tag=f"w3T{len(head_sb)}")
            for kh in range(2):
                nc.sync.dma_start(w3T[:, kh:kh + 1],
                                  w3[0, kh * 128:(kh + 1) * 128][:, None])
            b1c = P.tile([128, 2], F32, tag=f"b1c{len(head_sb)}")
            b2c = P.tile([128, 2], F32, tag=f"b2c{len(head_sb)}")
            for rh in range(2):
                nc.sync.dma_start(b1c[:, rh:rh + 1],
                                  b1[rh * 128:(rh + 1) * 128][:, None])
                nc.sync.dma_start(b2c[:, rh:rh + 1],
                                  b2[rh * 128:(rh + 1) * 128][:, None])
            b3c = P.tile([1, 1], F32, tag=f"b3c{len(head_sb)}")
            nc.sync.dma_start(b3c[:, :], b3[:][None, :])
            head_sb.append((w1T, w2T, w3T, b1c, b2c, b3c))

        ctxA.close()

        # ---------------- Phase C: combine + heads ----------------
        ps_c = ctx.enter_context(tc.tile_pool(name="ps_c", bufs=4, space="PSUM"))
        _phase_c(nc, tc, ctx, b_loc, P, scratch, ps_c, sg_raw, s_row,
                 ones_row, UwT, mT, Ub_row, wgT, saT, head_sb, out_d)

    nc.compile()
    return nc


def _phase_c(nc, tc, ctx, b_loc, P, scratch, ps_c, sg_raw, s_row,
             ones_row, UwT, mT, Ub_row, wgT, saT, head_sb, out_d):
    total = P.tile([1, b_loc], F32, tag="total")
    nc.vector.tensor_tensor(out=total[:, :], in0=sg_raw[:, :], in1=s_row[:, :],
                            op=AF.add)
    recip = P.tile([1, b_loc], F32, tag="recip")
    nc.vector.reciprocal_approx_fast(recip[:, :], total[:, :])
    gn_row = P.tile([1, b_loc], F32, tag="gn")
    nc.vector.tensor_tensor(out=gn_row[:, :], in0=sg_raw[:, :], in1=recip[:, :],
                            op=AF.mult)

    r32_ps = ps_c.tile([HID, b_loc], F32, tag="cps")
    nc.tensor.matmul(out=r32_ps[:, :], lhsT=ones_row[0:1, 0:HID], rhs=recip[:, :])
    r32 = P.tile([HID, b_loc], F32, tag="r32")
    nc.scalar.copy(r32[:, :], r32_ps[:, :])
    g32_ps = ps_c.tile([HID, b_loc], F32, tag="cps")
    nc.tensor.matmul(out=g32_ps[:, :], lhsT=ones_row[0:1, 0:HID], rhs=gn_row[:, :])
    g32 = P.tile([HID, b_loc], F32, tag="g32")
    nc.scalar.copy(g32[:, :], g32_ps[:, :])

    lT_ps = ps_c.tile([HID, b_loc], F32, tag="cps")
    nc.tensor.matmul(out=lT_ps[:, :], lhsT=UwT[:, :], rhs=mT[:, :],
                     start=True, stop=False)
    nc.tensor.matmul(out=lT_ps[:, :], lhsT=Ub_row[:, :], rhs=s_row[:, :],
                     start=False, stop=True)

    lnorm = P.tile([HID, b_loc], F32, tag="lnorm")
    nc.vector.tensor_tensor(out=lnorm[:, :], in0=lT_ps[:, :], in1=r32[:, :],
                            op=AF.mult)
    gpart = P.tile([HID, b_loc], F32, tag="gpart")
    nc.vector.tensor_tensor(out=gpart[:, :], in0=wgT[:, :], in1=g32[:, :],
                            op=AF.mult)
    nc.scalar.activation(saT[0:HID, :], gpart[:, :], RELU)
    nc.scalar.activation(saT[HID:2 * HID, :], lnorm[:, :], RELU)

    for h, (w1T, w2T, w3T, b1c, b2c, b3c) in enumerate(head_sb):
        h1 = []
        for rh in range(2):
            h_ps = ps_c.tile([128, b_loc], F32, tag="cps")
            nc.tensor.matmul(out=h_ps[:, :], lhsT=w1T[:, rh * 128:(rh + 1) * 128],
                             rhs=saT[:, :])
            h_sb = scratch.tile([128, b_loc], F32, tag="h1sb")
            nc.scalar.activation(h_sb[:, :], h_ps[:, :], RELU,
                                 bias=b1c[:, rh:rh + 1])
            h1.append(h_sb)
        h2 = []
        for rh in range(2):
            h_ps = ps_c.tile([128, b_loc], F32, tag="cps")
            for kh in range(2):
                nc.tensor.matmul(out=h_ps[:, :],
                                 lhsT=w2T[kh][:, rh * 128:(rh + 1) * 128],
                                 rhs=h1[kh][:, :],
                                 start=(kh == 0), stop=(kh == 1))
            h_sb = scratch.tile([128, b_loc], F32, tag="h2sb")
            nc.scalar.activation(h_sb[:, :], h_ps[:, :], RELU,
                                 bias=b2c[:, rh:rh + 1])
            h2.append(h_sb)
        q_ps = ps_c.tile([1, b_loc], F32, tag="cps")
        for kh in range(2):
            nc.tensor.matmul(out=q_ps[:, :], lhsT=w3T[:, kh:kh + 1],
                             rhs=h2[kh][:, :], start=(kh == 0), stop=(kh == 1))
        q_row = scratch.tile([1, b_loc], F32, tag="qrow")
        nc.scalar.activation(q_row[:, :], q_ps[:, :], IDENT, bias=b3c[:, :])
        nc.sync.dma_start(out_d[h:h + 1, :], q_row[:, :])


def _make_m16():
    m = np.zeros((128, SPC), np.float16)
    for p in range(128):
        m[p, p // PPS] = 1.0
    return m


def _make_esel():
    e = np.zeros((128, PPS * 128), np.float32)
    for r in range(PPS):
        for p in range(128):
            e[r * SPC + p // PPS, r * 128 + p] = 1.0
    return e


def _shard_inputs(inputs, b_loc=B_LOC):
    """Full inputs -> list of per-core in_maps."""
    m16 = _make_m16()
    esel = _make_esel()
    maps = []
    for c in range(NCORES):
        sl = slice(c * b_loc, (c + 1) * b_loc)
        m = {
            "local_states": np.ascontiguousarray(
                inputs["local_states"][sl].reshape(b_loc * L, L_DIM)),
            "global_states": np.ascontiguousarray(inputs["global_states"][sl]),
            "actions": np.ascontiguousarray(inputs["actions"][sl]),
            "m16": m16,
            "esel": esel,
        }
        for k in ("W_w", "W_b", "U_w", "U_b", "att_b",
                  "l1_w", "l1_b", "l2_w", "l2_b", "l3_w", "l3_b",
                  "l4_w", "l4_b", "l5_w", "l5_b", "l6_w", "l6_b"):
            m[k] = np.ascontiguousarray(np.asarray(inputs[k], np.float32))
        m["att_w"] = np.ascontiguousarray(
            np.asarray(inputs["att_w"], np.float32).reshape(1, 2 * HID))
        maps.append(m)
    return maps


_CACHE = {}


def kernel(**inputs) -> np.ndarray:
    from concourse.bass_utils import run_bass_kernel_spmd

    inputs = {k: np.asarray(v, np.float32) for k, v in inputs.items()}
    if "nc" not in _CACHE:
        _CACHE["nc"] = build_bass()
    nc = _CACHE["nc"]
    maps = _shard_inputs(inputs)
    res = run_bass_kernel_spmd(nc, maps, list(range(NCORES)))
    outs = [res.results[c]["out"] for c in range(NCORES)]  # each [2, B_LOC]
    q = np.concatenate(outs, axis=1)  # [2, B]
    return q.reshape(2, B, 1).astype(np.float32)


# revision 22
# speedup vs baseline: 1.0332x; 1.0064x over previous
"""Trainium2 Bass kernel for the Critic (gnn_message_passing) problem.

Math (per sample b):
  wg   = W_w @ g + W_b                                  [32]
  ul_l = U_w @ x_l + U_b                                [32]  (never materialized)
  score_l = lrelu(a1.wg + a2.ul_l + att_b) = lrelu(x_l . v + c_b)
        where v = U_w^T a2  (128-vec),  c_b = a1.wg + att_b + U_b.a2
  score_g = lrelu((a1+a2).wg + att_b)
  total = score_g + sum_l score_l
  l_part = (U_w @ m_b + U_b * s_b) / total   with m_b = sum_l score_l x_l, s_b = sum_l score_l
  g_part = (score_g / total) * wg
  sa = [relu(g_part); relu(l_part); action]            [128]
  q_h = l3 @ relu(l2 @ relu(l1 @ sa + b1) + b2) + b3   (two heads)

Layout (per core, pure data parallel x8, B_LOC = 512 samples):
  - local_states streamed in 32 fp32 chunks of 16 samples, [128 part, 25*128]:
    partition p holds 25 CONSECUTIVE tokens of sample p//8 (12.8KB contiguous
    descriptors). ACT makes an fp16 copy for the PE m-pass.
  - t = x.v stays fully fp32 (score errors amplify ~40x downstream): V-lane
    chunks run 25 fused TTR custom ops on DVE (c folded in as the accum
    init); G-lane chunks run one big GPSIMD multiply + one segmented DVE
    tensor_reduce + ACT bias add. Ratio tuned to balance DVE vs GPSIMD.
  - score split hi+lo (fp16 pair carries ~22 mantissa bits); m accumulated
    on PE with the x16 tile stationary and scorem [128, 2*16] moving:
    mT_chunk [128 feat, 2*16] in PSUM, folded hi+lo on copy-out.
  - s via two m16-stationary matmuls + ACT accumulate into s_colT [16, 32];
    one SBUF->SBUF DMA flattens to s_row [1, 512] after the loop.
  - Head MLPs run feature-major (transposed activations) on PE (fp32).
"""
import os
import sys

sys.path.insert(0, "/opt/trn_rl_repo")

from contextlib import ExitStack

import numpy as np

import concourse.bass as bass
import concourse.tile as tile
from concourse import bacc
from concourse import mybir
from concourse.dve_ops import TENSOR_TENSOR_REDUCE as CUSTOM_TTR

F32 = mybir.dt.float32
F16 = mybir.dt.float16
AF = mybir.AluOpType
IDENT = mybir.ActivationFunctionType.Identity
RELU = mybir.ActivationFunctionType.Relu

G_DIM, L_DIM, A_DIM, HID = 256, 128, 64, 32
B, L = 4096, 200
NCORES = 8
B_LOC = B // NCORES          # 512 samples per core
J = 25                       # tokens per partition per chunk
SPC = 16                     # samples per chunk (128 partitions / 8 per sample)
PPS = L // J                 # partitions per sample = 8
NCHUNK = B_LOC // SPC        # 32 chunks
NB = B_LOC // 128            # 128-sample blocks
V_LANES = 7                  # of every 16 chunks, this many use the DVE TTR lane


def build_bass(b_loc=B_LOC):
    tok = b_loc * L
    nc = bacc.Bacc()

    ls = nc.dram_tensor("local_states", [tok, L_DIM], F32, kind="ExternalInput")
    gs = nc.dram_tensor("global_states", [b_loc, G_DIM], F32, kind="ExternalInput")
    ac = nc.dram_tensor("actions", [b_loc, A_DIM], F32, kind="ExternalInput")
    Ww = nc.dram_tensor("W_w", [HID, G_DIM], F32, kind="ExternalInput")
    Wb = nc.dram_tensor("W_b", [HID], F32, kind="ExternalInput")
    Uw = nc.dram_tensor("U_w", [HID, L_DIM], F32, kind="ExternalInput")
    Ub = nc.dram_tensor("U_b", [HID], F32, kind="ExternalInput")
    attw = nc.dram_tensor("att_w", [1, 2 * HID], F32, kind="ExternalInput")
    attb = nc.dram_tensor("att_b", [1], F32, kind="ExternalInput")
    heads = []
    for h, names in enumerate((("l1", "l2", "l3"), ("l4", "l5", "l6"))):
        w1 = nc.dram_tensor(f"{names[0]}_w", [256, 128], F32, kind="ExternalInput")
        b1 = nc.dram_tensor(f"{names[0]}_b", [256], F32, kind="ExternalInput")
        w2 = nc.dram_tensor(f"{names[1]}_w", [256, 256], F32, kind="ExternalInput")
        b2 = nc.dram_tensor(f"{names[1]}_b", [256], F32, kind="ExternalInput")
        w3 = nc.dram_tensor(f"{names[2]}_w", [1, 256], F32, kind="ExternalInput")
        b3 = nc.dram_tensor(f"{names[2]}_b", [1], F32, kind="ExternalInput")
        heads.append((w1, b1, w2, b2, w3, b3))
    m16_d = nc.dram_tensor("m16", [128, SPC], F16, kind="ExternalInput")
    esel_d = nc.dram_tensor("esel", [128, PPS * 128], F32, kind="ExternalInput")
    out_d = nc.dram_tensor("out", [2, b_loc], F32, kind="ExternalOutput")

    with tile.TileContext(nc) as tc, ExitStack() as ctx:
        P = ctx.enter_context(tc.tile_pool(name="persist", bufs=1))
        scratch = ctx.enter_context(tc.tile_pool(name="scratch", bufs=2))
        ctxA = ctx.enter_context(ExitStack())
        ps_t = ctxA.enter_context(tc.tile_pool(name="ps_t", bufs=2, space="PSUM"))

        # ---------------- Phase A (lean): just what the stream needs --------
        from concourse.masks import make_identity

        ident = P.tile([128, 128], F32, tag="ident")
        make_identity(nc, ident[:, :])
        ones_row = P.tile([1, 128], F32, tag="onesr")
        nc.vector.memset(ones_row[:, :], 1.0)
        m16_sb = P.tile([128, SPC], F16, tag="m16")
        nc.sync.dma_start(m16_sb[:, :], m16_d[:, :])
        esel = P.tile([128, PPS * 128], F32, tag="esel")
        nc.sync.dma_start(esel[:, :], esel_d[:, :])

        def transpose_to_sbuf(dst_ap, src_ap):
            pp, ff = src_ap.shape
            t_ps = ps_t.tile([128, 128], F32, tag="tps")
            nc.tensor.transpose(t_ps[0:ff, 0:pp], src_ap, ident[0:pp, 0:pp])
            nc.scalar.copy(dst_ap, t_ps[0:ff, 0:pp])

        # small weights
        Ww_sb = P.tile([HID, G_DIM], F32, tag="Ww")
        nc.sync.dma_start(Ww_sb[:, :], Ww[:, :])
        Wb_sb = P.tile([HID, 1], F32, tag="Wb")
        nc.sync.dma_start(Wb_sb[:, :], Wb[:][:, None])
        Uw_sb = P.tile([HID, L_DIM], F32, tag="Uw")
        nc.sync.dma_start(Uw_sb[:, :], Uw[:, :])
        Ub_col = P.tile([HID, 1], F32, tag="Ubc")
        nc.sync.dma_start(Ub_col[:, :], Ub[:][:, None])
        Ub_row = P.tile([1, HID], F32, tag="Ubr")
        nc.sync.dma_start(Ub_row[:, :], Ub[:][None, :])
        a1_sb = P.tile([HID, 1], F32, tag="a1")
        nc.sync.dma_start(a1_sb[:, :], attw[0, 0:HID][:, None])
        a2_sb = P.tile([HID, 1], F32, tag="a2")
        nc.sync.dma_start(a2_sb[:, :], attw[0, HID:2 * HID][:, None])
        attb_sb = P.tile([1, 1], F32, tag="attb")
        nc.sync.dma_start(attb_sb[:, :], attb[:][None, :])

        # global_states (kept resident; also reused for gT transposes later)
        g_nat = []
        for bb in range(NB):
            g = P.tile([128, G_DIM], F32, tag=f"gnat{bb}")
            nc.sync.dma_start(g[:, :], gs[bb * 128:(bb + 1) * 128, :])
            g_nat.append(g)

        # v_rep [128, 128] fp32 = ones (x) (a2^T U_w)
        v_ps = ps_t.tile([1, L_DIM], F32, tag="tps")
        nc.tensor.matmul(out=v_ps[:, :], lhsT=a2_sb[:, :], rhs=Uw_sb[:, :])
        v_row = P.tile([1, L_DIM], F32, tag="vrow")
        nc.scalar.copy(v_row[:, :], v_ps[:, :])
        vrep_ps = ps_t.tile([128, 128], F32, tag="tps")
        nc.tensor.matmul(out=vrep_ps[:, :], lhsT=ones_row[:, :], rhs=v_row[:, :])
        v_rep = P.tile([128, 128], F32, tag="vrep")
        nc.scalar.copy(v_rep[:, :], vrep_ps[:, :])

        # c_col4 [128, NB]: c (minus const) per sample, sample-partition,
        # via u = W_w^T a1 then per-block STT free-dim accumulation.
        u_ps = ps_t.tile([1, G_DIM], F32, tag="tps")
        nc.tensor.matmul(out=u_ps[:, :], lhsT=a1_sb[:, :], rhs=Ww_sb[:, :])
        u_row = P.tile([1, G_DIM], F32, tag="urow")
        nc.scalar.copy(u_row[:, :], u_ps[:, :])
        urep_ps = ps_t.tile([128, G_DIM], F32, tag="tps")
        nc.tensor.matmul(out=urep_ps[:, :], lhsT=ones_row[:, :], rhs=u_row[:, :])
        u_rep = P.tile([128, G_DIM], F32, tag="urep")
        nc.scalar.copy(u_rep[:, :], urep_ps[:, :])
        c_col4 = P.tile([128, NB], F32, tag="ccol4")
        junkA = scratch.tile([128, G_DIM], F32, tag="junkA")
        for bb in range(NB):
            nc.vector._custom_dve(
                CUSTOM_TTR, out=junkA[:, :], in0=g_nat[bb][:, :],
                in1=u_rep[:, :], s0=0.0, s1=1.0,
                accum_out=c_col4[:, bb:bb + 1])

        # cconst = U_b.a2 + att_b + a1.W_b, broadcast to 128 partitions
        uba2_ps = ps_t.tile([1, 1], F32, tag="tps")
        nc.tensor.matmul(out=uba2_ps[:, :], lhsT=Ub_col[:, :], rhs=a2_sb[:, :],
                         start=True, stop=False, skip_group_check=True)
        nc.tensor.matmul(out=uba2_ps[:, :], lhsT=Wb_sb[:, :], rhs=a1_sb[:, :],
                         start=False, stop=True, skip_group_check=True)
        cconst = P.tile([1, 1], F32, tag="cconst")
        nc.vector.tensor_tensor(out=cconst[:, :], in0=uba2_ps[:, :],
                                in1=attb_sb[:, :], op=AF.add)
        cc128_ps = ps_t.tile([128, 1], F32, tag="tps")
        nc.tensor.matmul(out=cc128_ps[:, :], lhsT=ones_row[0:1, :],
                         rhs=cconst[:, :])
        cc128 = P.tile([128, 1], F32, tag="cc128")
        nc.scalar.copy(cc128[:, :], cc128_ps[:, :])

        # c_all [128, NCHUNK]: column ch holds c[ch*16 + p//8].
        # esel[:, r*128:(r+1)*128] is E_r with E_r[q, p] = ind(q == r*16+p//8);
        # c_all[:, r::PPS] = E_r^T @ c_col4.
        call_ps = ps_t.tile([128, NCHUNK], F32, tag="tps")
        for r in range(PPS):
            nc.tensor.matmul(out=call_ps[:, r:NCHUNK:PPS],
                             lhsT=esel[:, r * 128:(r + 1) * 128],
                             rhs=c_col4[:, :], skip_group_check=True)
        c_all = P.tile([128, NCHUNK], F32, tag="call")
        nc.scalar.activation(c_all[:, :], call_ps[:, :], IDENT,
                             bias=cc128[:, :])

        # ---------------- Phase B: main token stream ----------------
        ctxB = ctx.enter_context(ExitStack())
        xpool = ctx.enter_context(tc.tile_pool(name="xchunk", bufs=4))
        x16pool = ctx.enter_context(tc.tile_pool(name="x16", bufs=3))
        ppool = ctx.enter_context(tc.tile_pool(name="prod", bufs=3))
        tpool = ctx.enter_context(tc.tile_pool(name="tbuf", bufs=3))
        jpool = ctx.enter_context(tc.tile_pool(name="junk", bufs=2))
        smpool = ctx.enter_context(tc.tile_pool(name="scorem", bufs=3))
        ps_m = ctxB.enter_context(tc.tile_pool(name="ps_m", bufs=2, space="PSUM"))
        ps_s = ctxB.enter_context(tc.tile_pool(name="ps_s", bufs=2, space="PSUM"))

        mT = P.tile([L_DIM, b_loc], F32, tag="mT")
        s_colT = P.tile([SPC, NCHUNK], F32, tag="scolT")

        GP_SLOTS = {1, 2, 4, 5, 7, 8, 10, 11, 13}  # 9 of 16 multiplies on GPSIMD
        for ch in range(NCHUNK):
            use_v = (ch % 16) in GP_SLOTS
            x_ch = xpool.tile([128, J * 128], F32, tag="xch")
            src = ls[ch * J * 128:(ch + 1) * J * 128, :]
            nc.sync.dma_start(
                x_ch[:, :], src.rearrange("(p j) d -> p (j d)", p=128))
            x16 = x16pool.tile([128, J * 128], F16, tag="x16")
            nc.scalar.copy(x16[:, :], x_ch[:, :])

            # z[p, j] = sum_d x[p,j,d]*v[d] + c  (all fp32); the multiply
            # alternates GPSIMD/DVE to balance engine load, reduce on DVE
            prod = ppool.tile([128, J * 128], F32, tag="pr")
            mult_eng = nc.gpsimd if use_v else nc.vector
            mult_eng.tensor_tensor(
                out=prod[:, :].rearrange("p (j d) -> p j d", d=128),
                in0=x_ch[:, :].rearrange("p (j d) -> p j d", d=128),
                in1=v_rep[:, None, :].broadcast_to((128, J, 128)),
                op=AF.mult)
            t_raw = tpool.tile([128, J], F32, tag="traw")
            nc.vector.tensor_reduce(
                out=t_raw[:, :],
                in_=prod[:, :].rearrange("p (j d) -> p j d", d=128),
                axis=mybir.AxisListType.X, op=AF.add)
            # z/score/hi/lo all stay on the DVE queue — no cross-engine hops
            z = tpool.tile([128, J], F32, tag="z")
            nc.vector.tensor_scalar_add(z[:, :], t_raw[:, :],
                                        c_all[:, ch:ch + 1])
            score = tpool.tile([128, J], F32, tag="sc")
            nc.vector.scalar_tensor_tensor(
                out=score[:, :], in0=z[:, :], scalar=0.01, in1=z[:, :],
                op0=AF.mult, op1=AF.max)
            # split score = hi + lo (fp16 pair ~= 22 mantissa bits)
            hi = tpool.tile([128, J], F16, tag="hi")
            nc.vector.tensor_copy(hi[:, :], score[:, :])
            lo = tpool.tile([128, J], F16, tag="lo")
            nc.vector.tensor_tensor(out=lo[:, :], in0=score[:, :],
                                    in1=hi[:, :], op=AF.subtract)
            scorem = smpool.tile([128, J * 2 * SPC], F16, tag="sm")
            sm3 = scorem[:, :].rearrange("p (j t s) -> p j t s", t=2, s=SPC)
            nc.gpsimd.tensor_tensor(
                out=sm3[:, :, 0, :],
                in0=hi[:, :, None].broadcast_to((128, J, SPC)),
                in1=m16_sb[:, None, :].broadcast_to((128, J, SPC)),
                op=AF.mult)
            nc.gpsimd.tensor_tensor(
                out=sm3[:, :, 1, :],
                in0=lo[:, :, None].broadcast_to((128, J, SPC)),
                in1=m16_sb[:, None, :].broadcast_to((128, J, SPC)),
                op=AF.mult)

            # mT_chunk [128 feat, 2*16] accumulated over the 25 tiles
            mT_ps = ps_m.tile([L_DIM, 2 * SPC], F32, tag="mps")
            for j in range(J):
                nc.tensor.matmul(out=mT_ps[:, :],
                                 lhsT=x16[:, j * 128:(j + 1) * 128],
                                 rhs=scorem[:, j * 2 * SPC:(j + 1) * 2 * SPC],
                                 start=(j == 0), stop=(j == J - 1))
            nc.scalar.copy(mT[:, ch * SPC:(ch + 1) * SPC], mT_ps[:, 0:SPC])
            nc.vector.tensor_tensor(out=mT[:, ch * SPC:(ch + 1) * SPC],
                                    in0=mT_ps[:, SPC:2 * SPC],
                                    in1=mT[:, ch * SPC:(ch + 1) * SPC],
                                    op=AF.add)

            # s[s] = sum_{p,j} (hi+lo)[p,j]*ind(p//8==s)  -> s_colT[:, ch]
            s_ps = ps_s.tile([SPC, J], F32, tag="sps")
            nc.tensor.matmul(out=s_ps[:, :], lhsT=m16_sb[:, :], rhs=hi[:, :],
                             start=True, stop=False)
            nc.tensor.matmul(out=s_ps[:, :], lhsT=m16_sb[:, :], rhs=lo[:, :],
                             start=False, stop=True)
            junk_s = jpool.tile([SPC, J], F32, tag="jks")
            nc.scalar.activation(junk_s[:, :], s_ps[:, :], IDENT,
                                 accum_out=s_colT[:, ch:ch + 1])

        # flatten s_colT [16, 32] -> s_row [1, 512] (order: sample = ch*16+s)
        # via 16 selection matmuls into one PSUM row (strided col slices)
        srow_ps = ps_s.tile([1, b_loc], F32, tag="srps")
        for s in range(SPC):
            nc.tensor.matmul(out=srow_ps[0:1, s * NCHUNK:(s + 1) * NCHUNK],
                             lhsT=ident[0:SPC, s:s + 1], rhs=s_colT[:, :],
                             skip_group_check=True)
        s_row = P.tile([1, b_loc], F32, tag="srow")
        nc.scalar.copy(
            s_row[0:1, :].rearrange("one (c s) -> one c s", s=SPC),
            srow_ps[0:1, :].rearrange("one (s c) -> one c s", s=SPC))

        ctxB.close()

        # ------------- Phase A tail (overlaps the stream) -------------------
        # gT transposes, wgT, sg_raw, saT/actions, head weights: consumed only
        # by phase C, so they schedule behind the chunk traffic.
        gT = []
        for g in range(G_DIM // 128):
            t = P.tile([128, b_loc], F32, tag=f"gT{g}")
            gT.append(t)
        for bb in range(NB):
            for g in range(G_DIM // 128):
                transpose_to_sbuf(gT[g][:, bb * 128:(bb + 1) * 128],
                                  g_nat[bb][:, g * 128:(g + 1) * 128])
        WwT = []
        for g in range(G_DIM // 128):
            w = P.tile([128, HID], F32, tag=f"WwT{g}")
            transpose_to_sbuf(w[:, :], Ww_sb[:, g * 128:(g + 1) * 128])
            WwT.append(w)
        UwT = P.tile([L_DIM, HID], F32, tag="UwT")
        transpose_to_sbuf(UwT[:, :], Uw_sb[:, :])

        wgT_ps = ps_t.tile([HID, b_loc], F32, tag="tps")
        for g in range(G_DIM // 128):
            nc.tensor.matmul(out=wgT_ps[:, :], lhsT=WwT[g][:, :], rhs=gT[g][:, :],
                             start=(g == 0), stop=(g == G_DIM // 128 - 1))
        wgT = P.tile([HID, b_loc], F32, tag="wgT")
        nc.scalar.activation(wgT[:, :], wgT_ps[:, :], IDENT, bias=Wb_sb[:, :])

        a12 = P.tile([HID, 1], F32, tag="a12")
        nc.vector.tensor_tensor(out=a12[:, :], in0=a1_sb[:, :], in1=a2_sb[:, :],
                                op=AF.add)
        sg_ps = ps_t.tile([1, b_loc], F32, tag="tps")
        nc.tensor.matmul(out=sg_ps[:, :], lhsT=a12[:, :], rhs=wgT[:, :])
        sg_lin = P.tile([1, b_loc], F32, tag="sg_lin")
        nc.scalar.activation(sg_lin[:, :], sg_ps[:, :], IDENT, bias=attb_sb[:, :])
        sg_raw = P.tile([1, b_loc], F32, tag="sg_raw")
        nc.vector.scalar_tensor_tensor(out=sg_raw[:, :], in0=sg_lin[:, :],
                                       scalar=0.01, in1=sg_lin[:, :],
                                       op0=AF.mult, op1=AF.max)

        saT = P.tile([128, b_loc], F32, tag="saT")
        for bb in range(NB):
            a_nat = scratch.tile([128, A_DIM], F32, tag="anat")
            nc.sync.dma_start(a_nat[:, :], ac[bb * 128:(bb + 1) * 128, :])
            transpose_to_sbuf(saT[2 * HID:2 * HID + A_DIM, bb * 128:(bb + 1) * 128],
                              a_nat[:, :])

        head_sb = []
        for (w1, b1, w2, b2, w3, b3) in heads:
            w1_nat = scratch.tile([128, 128], F32, tag="w1nat")
            w1T = P.tile([128, 256], F32, tag=f"w1T{len(head_sb)}")
            for rh in range(2):
                nc.sync.dma_start(w1_nat[:, :], w1[rh * 128:(rh + 1) * 128, :])
                transpose_to_sbuf(w1T[:, rh * 128:(rh + 1) * 128], w1_nat[:, :])
            w2T = [P.tile([128, 256], F32, tag=f"w2T{len(head_sb)}_{kh}",
                          name=f"w2T{len(head_sb)}_{kh}")
                   for kh in range(2)]
            for rh in range(2):
                for kh in range(2):
                    w2_nat = scratch.tile([128, 128], F32, tag="w2nat")
                    nc.sync.dma_start(
                        w2_nat[:, :],
                        w2[rh * 128:(rh + 1) * 128, kh * 128:(kh + 1) * 128])
                    transpose_to_sbuf(w2T[kh][:, rh * 128:(rh + 1) * 128],
                                      w2_nat[:, :])
            w3T = P.tile([128, 2], F32, tag=f"w3T{len(head_sb)}")
            for kh in range(2):
                nc.sync.dma_start(w3T[:, kh:kh + 1],
                                  w3[0, kh * 128:(kh + 1) * 128][:, None])
            b1c = P.tile([128, 2], F32, tag=f"b1c{len(head_sb)}")
            b2c = P.tile([128, 2], F32, tag=f"b2c{len(head_sb)}")
            for rh in range(2):
                nc.sync.dma_start(b1c[:, rh:rh + 1],
                                  b1[rh * 128:(rh + 1) * 128][:, None])
                nc.sync.dma_start(b2c[:, rh:rh + 1],
                                  b2[rh * 128:(rh + 1) * 128][:, None])
            b3c = P.tile([1, 1], F32, tag=f"b3c{len(head_sb)}")
            nc.sync.dma_start(b3c[:, :], b3[:][None, :])
            head_sb.append((w1T, w2T, w3T, b1c, b2c, b3c))

        ctxA.close()

        # ---------------- Phase C: combine + heads ----------------
        ps_c = ctx.enter_context(tc.tile_pool(name="ps_c", bufs=4, space="PSUM"))
        _phase_c(nc, tc, ctx, b_loc, P, scratch, ps_c, sg_raw, s_row,
                 ones_row, UwT, mT, Ub_row, wgT, saT, head_sb, out_d)

    nc.compile()
    return nc


def _phase_c(nc, tc, ctx, b_loc, P, scratch, ps_c, sg_raw, s_row,
             ones_row, UwT, mT, Ub_row, wgT, saT, head_sb, out_d):
    total = P.tile([1, b_loc], F32, tag="total")
    nc.vector.tensor_tensor(out=total[:, :], in0=sg_raw[:, :], in1=s_row[:, :],
                            op=AF.add)
    recip = P.tile([1, b_loc], F32, tag="recip")
    nc.vector.reciprocal_approx_fast(recip[:, :], total[:, :])
    gn_row = P.tile([1, b_loc], F32, tag="gn")
    nc.vector.tensor_tensor(out=gn_row[:, :], in0=sg_raw[:, :], in1=recip[:, :],
                            op=AF.mult)

    r32_ps = ps_c.tile([HID, b_loc], F32, tag="cps")
    nc.tensor.matmul(out=r32_ps[:, :], lhsT=ones_row[0:1, 0:HID], rhs=recip[:, :])
    r32 = P.tile([HID, b_loc], F32, tag="r32")
    nc.scalar.copy(r32[:, :], r32_ps[:, :])
    g32_ps = ps_c.tile([HID, b_loc], F32, tag="cps")
    nc.tensor.matmul(out=g32_ps[:, :], lhsT=ones_row[0:1, 0:HID], rhs=gn_row[:, :])
    g32 = P.tile([HID, b_loc], F32, tag="g32")
    nc.scalar.copy(g32[:, :], g32_ps[:, :])

    lT_ps = ps_c.tile([HID, b_loc], F32, tag="cps")
    nc.tensor.matmul(out=lT_ps[:, :], lhsT=UwT[:, :], rhs=mT[:, :],
                     start=True, stop=False)
    nc.tensor.matmul(out=lT_ps[:, :], lhsT=Ub_row[:, :], rhs=s_row[:, :],
                     start=False, stop=True)

    lnorm = P.tile([HID, b_loc], F32, tag="lnorm")
    nc.vector.tensor_tensor(out=lnorm[:, :], in0=lT_ps[:, :], in1=r32[:, :],
                            op=AF.mult)
    gpart = P.tile([HID, b_loc], F32, tag="gpart")
    nc.vector.tensor_tensor(out=gpart[:, :], in0=wgT[:, :], in1=g32[:, :],
                            op=AF.mult)
    nc.scalar.activation(saT[0:HID, :], gpart[:, :], RELU)
    nc.scalar.activation(saT[HID:2 * HID, :], lnorm[:, :], RELU)

    for h, (w1T, w2T, w3T, b1c, b2c, b3c) in enumerate(head_sb):
        h1 = []
        for rh in range(2):
            h_ps = ps_c.tile([128, b_loc], F32, tag="cps")
            nc.tensor.matmul(out=h_ps[:, :], lhsT=w1T[:, rh * 128:(rh + 1) * 128],
                             rhs=saT[:, :])
            h_sb = scratch.tile([128, b_loc], F32, tag="h1sb")
            nc.scalar.activation(h_sb[:, :], h_ps[:, :], RELU,
                                 bias=b1c[:, rh:rh + 1])
            h1.append(h_sb)
        h2 = []
        for rh in range(2):
            h_ps = ps_c.tile([128, b_loc], F32, tag="cps")
            for kh in range(2):
                nc.tensor.matmul(out=h_ps[:, :],
                                 lhsT=w2T[kh][:, rh * 128:(rh + 1) * 128],
                                 rhs=h1[kh][:, :],
                                 start=(kh == 0), stop=(kh == 1))
            h_sb = scratch.tile([128, b_loc], F32, tag="h2sb")
            nc.scalar.activation(h_sb[:, :], h_ps[:, :], RELU,
                                 bias=b2c[:, rh:rh + 1])
            h2.append(h_sb)
        q_ps = ps_c.tile([1, b_loc], F32, tag="cps")
        for kh in range(2):
            nc.tensor.matmul(out=q_ps[:, :], lhsT=w3T[:, kh:kh + 1],
                             rhs=h2[kh][:, :], start=(kh == 0), stop=(kh == 1))
        q_row = scratch.tile([1, b_loc], F32, tag="qrow")
        nc.scalar.activation(q_row[:, :], q_ps[:, :], IDENT, bias=b3c[:, :])
        nc.sync.dma_start(out_d[h:h + 1, :], q_row[:, :])


def _make_m16():
    m = np.zeros((128, SPC), np.float16)
    for p in range(128):
        m[p, p // PPS] = 1.0
    return m


def _make_esel():
    e = np.zeros((128, PPS * 128), np.float32)
    for r in range(PPS):
        for p in range(128):
            e[r * SPC + p // PPS, r * 128 + p] = 1.0
    return e


def _shard_inputs(inputs, b_loc=B_LOC):
    """Full inputs -> list of per-core in_maps."""
    m16 = _make_m16()
    esel = _make_esel()
    maps = []
    for c in range(NCORES):
        sl = slice(c * b_loc, (c + 1) * b_loc)
        m = {
            "local_states": np.ascontiguousarray(
                inputs["local_states"][sl].reshape(b_loc * L, L_DIM)),
            "global_states": np.ascontiguousarray(inputs["global_states"][sl]),
            "actions": np.ascontiguousarray(inputs["actions"][sl]),
            "m16": m16,
            "esel": esel,
        }
        for k in ("W_w", "W_b", "U_w", "U_b", "att_b",
                  "l1_w", "l1_b", "l2_w", "l2_b", "l3_w", "l3_b",
                  "l4_w", "l4_b", "l5_w", "l5_b", "l6_w", "l6_b"):
            m[k] = np.ascontiguousarray(np.asarray(inputs[k], np.float32))
        m["att_w"] = np.ascontiguousarray(
            np.asarray(inputs["att_w"], np.float32).reshape(1, 2 * HID))
        maps.append(m)
    return maps


_CACHE = {}


def kernel(**inputs) -> np.ndarray:
    from concourse.bass_utils import run_bass_kernel_spmd

    inputs = {k: np.asarray(v, np.float32) for k, v in inputs.items()}
    if "nc" not in _CACHE:
        _CACHE["nc"] = build_bass()
    nc = _CACHE["nc"]
    maps = _shard_inputs(inputs)
    res = run_bass_kernel_spmd(nc, maps, list(range(NCORES)))
    outs = [res.results[c]["out"] for c in range(NCORES)]  # each [2, B_LOC]
    q = np.concatenate(outs, axis=1)  # [2, B]
    return q.reshape(2, B, 1).astype(np.float32)
